# revision 8
# baseline (speedup 1.0000x reference)
"""KoLeo loss kernel for Trainium2 (8 NeuronCores, data-parallel rows).

reference semantics:
    x = l2_normalize(student_output)            # [B, D]
    dots = x @ x.T ; dots[i, i] = -1
    I = argmax(dots, 1)
    loss = -mean(log(||x - x[I] + eps|| + eps))

Since rows are unit-norm, ||x_i - x_j|| = sqrt(2 - 2 * dot(x_i, x_j)), so
    loss = -0.5 * mean(ln(2 - 2 * max_j!=i dots[i, j]))
(the eps terms contribute ~1e-8 relative and are dropped).

Sharding: each core gets the full x^T (bf16, host-cast), column-rotated so its
own 1024 rows come first, computes its [1024, 8192] slice of the gram matrix
in fp8-e4m3 with DoubleRow matmuls (2 K-planes per instruction), and reduces
to a scalar partial sum of ln(2 - 2*maxdot). Host sums the 8 partials.

The row-max drain is the second-largest cost after the matmuls (8.4M PSUM
f32 reads/core; DVE and ACT are the only PSUM readers, both 1 elem/cycle/
lane).  It is split:
  - 2 of 8 column groups: exact DVE reduce_max from PSUM.
  - 6 of 8: ACT smooth-max -- one fused Exp activation per PSUM tile with
    accum_out summing exp(beta*(dot-c)) along the row; the per-row max is
    recovered as c + ln(sum)/beta (beta=256, c=0.22: error ~1e-4 vs exact,
    and exp(256*(-64-c)) flushes to zero so the diagonal kill is free).
This puts ~55us of drain on the otherwise-idle ACT and keeps DVE under the
PE's ~75us of matmul streaming.

Other structure per 1024-wide column group:
  1. DMA x^T bf16 (host pre-casts f32->bf16: halves HBM traffic)
  2. xsq = x*x bf16 (GpSimd; DVE for group 0 to shorten the startup chain)
  3. column norms via bf16 ones-matmul (PE broadcasts sums across
     partitions); inv = exp(-0.5*ln(norm2)) (ACT; the act-table universe is
     pinned to natural_log_exp_and_others => exactly one ACT_TABLE_LOAD)
  4. normalize: xq = x * inv, bf16*bf16 -> fp8 DoubleRow planes (DVE)
  5. gram: per row-tile one [128,1024] PSUM tile, 4 DoubleRow matmuls
     (K=256 each); diagonal killed by one extra bf16 I.T@(-64 shifted)
  6. drain as above; epilogue folds smooth+direct maxes, ln(2-2*max),
     row-sum, partition-sum via f32 ones-matmul, scalar partial out
"""

import numpy as np
import ml_dtypes

import concourse.bacc as bacc
import concourse.hw_specs as hw_specs
import concourse.tile as tile
from concourse import mybir
from concourse.bass_utils import run_bass_kernel_spmd

B, D = 8192, 512
N_CORES = 8
ROWS = B // N_CORES          # 1024 rows per core
P = 128                      # SBUF partitions
KT = D // P                  # 4 contraction k-tiles
KB = KT // 2                 # 2 DoubleRow k-blocks (2 planes each)
M_TILES = ROWS // P          # 8 output row tiles
NT = 512                     # matmul moving free dim (psum bank)
GW = 1024                    # column-group width == gram PSUM tile (2 banks)
NG = B // GW                 # 8 column groups
DIAG_C = 64.0                # diagonal kill constant
N_WARM = 40                  # PE warm-up matmuls issued under the input DMA

BETA = 256.0                 # smooth-max sharpness
CSHIFT = 0.22                # smooth-max recentering constant
DIRECT_NG = (0, 4)           # column groups drained by exact DVE reduce_max
N_DIRECT = len(DIRECT_NG)
N_EXP = NG - N_DIRECT
EXP_IDX = {ng: i for i, ng in enumerate(n for n in range(NG) if n not in DIRECT_NG)}

F32 = mybir.dt.float32
BF16 = mybir.dt.bfloat16
FP8 = mybir.dt.float8e4
AF = mybir.ActivationFunctionType
ALU = mybir.AluOpType
DR = mybir.MatmulPerfMode.DoubleRow

_CACHE: dict = {}


def _pin_act_tables():
    """Restrict the activation-table universe to natural_log_exp_and_others
    (contains ln, exp, copy, square, identity) so the table-load inserter
    emits exactly one ACT_TABLE_LOAD instead of thrashing ln<->exp sets.
    Set positions are preserved so the emitted act_func_set_id still indexes
    act_info.json correctly."""
    orig = hw_specs.get_activation_tables("gen3")
    pinned = {
        name: (fns if name == "natural_log_exp_and_others" else set())
        for name, fns in orig.items()
    }
    bacc.get_activation_tables = lambda arch: pinned


def _build():
    _pin_act_tables()
    nc = bacc.Bacc(
        "TRN2", target_bir_lowering=False, debug=False, num_devices=N_CORES
    )
    xt = nc.declare_dram_parameter("xt", [D, B], BF16, isOutput=False)
    ident = nc.declare_dram_parameter("ident", [P, P], BF16, isOutput=False)
    # ebig[p, 384 + p] = -DIAG_C, zero elsewhere; slicing [384-off : 896-off]
    # yields a [P, NT] tile with -DIAG_C at [p, off + p]
    ebig = nc.declare_dram_parameter("ebig", [P, NT + 3 * P], BF16, isOutput=False)
    partial = nc.declare_dram_parameter("partial", [1, 1], F32, isOutput=True)

    with tile.TileContext(nc) as tc:
        with (
            tc.tile_pool(name="big", bufs=1) as big,
            tc.tile_pool(name="work", bufs=2) as work,
            tc.tile_pool(name="sq", bufs=2) as sqp,
            tc.tile_pool(name="scr", bufs=3) as scrp,
            tc.tile_pool(name="small", bufs=2) as small,
        ):
            ident_sb = big.tile([P, P], BF16, name="ident_sb", tag="ident_sb")
            ebig_sb = big.tile([P, NT + 3 * P], BF16, name="ebig_sb", tag="ebig_sb")
            ones_sb = big.tile([P, P], BF16, name="ones_sb", tag="ones_sb")
            onesf_sb = big.tile([P, 1], F32, name="onesf_sb", tag="onesf_sb")
            nc.sync.dma_start(ident_sb[:], ident[:])
            nc.sync.dma_start(ebig_sb[:], ebig[:])
            nc.gpsimd.memset(ones_sb[:], 1.0)
            nc.gpsimd.memset(onesf_sb[:], 1.0)
            two_sb = small.tile([P, 1], F32, name="two_sb", tag="two_sb")
            nc.gpsimd.memset(two_sb[:], 2.0)
            # exp bias: exp(BETA*dot - BETA*CSHIFT)
            ebias_sb = small.tile([P, 1], F32, name="ebias_sb", tag="ebias_sb")
            nc.gpsimd.memset(ebias_sb[:], -BETA * CSHIFT)

            xbf = [
                big.tile([P, B], BF16, name=f"xbf{k}", tag=f"xbf{k}")
                for k in range(KT)
            ]
            xq = [
                big.tile([P, 2, B], FP8, name=f"xq{kb}", tag=f"xq{kb}")
                for kb in range(KB)
            ]
            invb = big.tile([P, B], BF16, name="invb", tag="invb")
            loglist = small.tile([P, M_TILES], F32, name="loglist", tag="loglist")
            # per (mi, direct group | smooth) row-max candidates
            maxall = small.tile(
                [P, M_TILES * (N_DIRECT + 1)], F32, name="maxall", tag="maxall"
            )
            # per (mi, exp group) partial exp-sums
            expall = small.tile(
                [P, M_TILES * N_EXP], F32, name="expall", tag="expall"
            )

            with tc.tile_pool(name="gpsum", bufs=4, space="PSUM") as gpsum:
                # PE warm-up: keep the HAM activity window busy during the
                # initial DMA so gram matmuls run at 2.4 GHz from the start.
                warm = gpsum.tile([P, GW], F32, name="warm", tag="g")
                for _ in range(N_WARM):
                    nc.tensor.matmul(
                        warm[:, 0:P], ident_sb[:], ident_sb[:], start=True, stop=True
                    )

                for ng in range(NG):
                    ns = slice(ng * GW, (ng + 1) * GW)
                    for k in range(KT):
                        nc.gpsimd.dma_start(
                            xbf[k][:, ns], xt[k * P : (k + 1) * P, ns]
                        )
                    # squared entries (bf16); GpSimd owns this except for the
                    # first group, which sits on the critical startup path
                    xsq = [
                        sqp.tile([P, GW], BF16, name=f"xsq_{ng}_{k}", tag=f"xsq{k}")
                        for k in range(KT)
                    ]
                    sq_eng = nc.vector if ng == 0 else nc.gpsimd
                    for k in range(KT):
                        sq_eng.tensor_mul(xsq[k][:], xbf[k][:, ns], xbf[k][:, ns])
                    # column norms broadcast across partitions via ones-matmul
                    nps = gpsum.tile([P, GW], F32, name="nps", tag="g")
                    for c in range(GW // NT):
                        for k in range(KT):
                            nc.tensor.matmul(
                                nps[:, c * NT : (c + 1) * NT],
                                ones_sb[:],
                                xsq[k][:, c * NT : (c + 1) * NT],
                                start=(k == 0),
                                stop=(k == KT - 1),
                            )
                    # inv = exp(-0.5*ln(norm2)); one pinned table set
                    lntmp = work.tile([P, GW], F32, name="lntmp", tag="lntmp")
                    nc.scalar.activation(lntmp[:], nps[:], AF.Ln)
                    nc.scalar.activation(
                        invb[:, ns], lntmp[:], AF.Exp, scale=-0.5
                    )
                    # normalize into fp8 DoubleRow planes: xq = x * inv
                    for k in range(KT):
                        nc.vector.tensor_mul(
                            xq[k // 2][:, k % 2, ns], xbf[k][:, ns], invb[:, ns]
                        )
                    # gram slice rows x this column group, then row-max drain
                    for mi in range(M_TILES):
                        g = gpsum.tile([P, GW], F32, name="g", tag="g")
                        # diag block for row-tile mi sits at columns
                        # [mi*128, mi*128+128) -- always group 0
                        diag_here = ng == 0
                        diag_c = (mi * P) // NT
                        for kb in range(KB):
                            for c in range(GW // NT):
                                c0 = ng * GW + c * NT
                                nc.tensor.matmul(
                                    g[:, c * NT : (c + 1) * NT],
                                    xq[kb][:, :, mi * P : (mi + 1) * P],
                                    xq[kb][:, :, c0 : c0 + NT],
                                    start=(kb == 0),
                                    stop=(
                                        kb == KB - 1
                                        and not (diag_here and c == diag_c)
                                    ),
                                    perf_mode=DR,
                                )
                        if diag_here:
                            off = (mi * P) % NT
                            # adds -DIAG_C at diag position [p, off+p]
                            nc.tensor.matmul(
                                g[:, diag_c * NT : (diag_c + 1) * NT],
                                ident_sb[:],
                                ebig_sb[:, 3 * P - off : 3 * P - off + NT],
                                start=False,
                                stop=True,
                            )
                        if ng in DIRECT_NG:
                            di = DIRECT_NG.index(ng)
                            col = mi * (N_DIRECT + 1) + di
                            nc.vector.reduce_max(
                                maxall[:, col : col + 1],
                                g[:],
                                axis=mybir.AxisListType.X,
                            )
                        else:
                            # fused smooth-max drain on ACT: accum_out sums
                            # exp(BETA*(dot - CSHIFT)) along the row
                            scr = scrp.tile([P, GW], BF16, name="scr", tag="scr")
                            col = mi * N_EXP + EXP_IDX[ng]
                            nc.scalar.activation(
                                scr[:],
                                g[:],
                                AF.Exp,
                                bias=ebias_sb[:],
                                scale=BETA,
                                accum_out=expall[:, col : col + 1],
                            )

                for mi in range(M_TILES):
                    # smooth max = CSHIFT + ln(sum of exp sums)/BETA
                    acc = small.tile([P, 1], F32, name="acc", tag="acc")
                    nc.vector.reduce_sum(
                        acc[:],
                        expall[:, mi * N_EXP : (mi + 1) * N_EXP],
                        axis=mybir.AxisListType.X,
                    )
                    acc2 = small.tile([P, 1], F32, name="acc2", tag="acc2")
                    nc.vector.tensor_scalar_max(acc2[:], acc[:], 1e-35)
                    lnacc = small.tile([P, 1], F32, name="lnacc", tag="lnacc")
                    nc.scalar.activation(lnacc[:], acc2[:], AF.Ln)
                    mcol = mi * (N_DIRECT + 1) + N_DIRECT
                    nc.vector.tensor_scalar(
                        maxall[:, mcol : mcol + 1],
                        lnacc[:],
                        1.0 / BETA,
                        CSHIFT,
                        op0=ALU.mult,
                        op1=ALU.add,
                    )
                    rowmax = small.tile([P, 1], F32, name="rowmax", tag="rowmax")
                    nc.vector.reduce_max(
                        rowmax[:],
                        maxall[:, mi * (N_DIRECT + 1) : (mi + 1) * (N_DIRECT + 1)],
                        axis=mybir.AxisListType.X,
                    )
                    # ln(2 - 2*maxdot) = 2*ln(nearest-neighbor distance)
                    nc.scalar.activation(
                        loglist[:, mi : mi + 1],
                        rowmax[:],
                        AF.Ln,
                        bias=two_sb[:],
                        scale=-2.0,
                    )

                # --- final reduction to one scalar per core ---
                sumlog = small.tile([P, 1], F32, name="sumlog", tag="sumlog")
                nc.vector.reduce_sum(
                    sumlog[:], loglist[:], axis=mybir.AxisListType.X
                )
                # partition sum via f32 matmul: [1,1] = sumlog.T @ ones
                tot = gpsum.tile([P, GW], F32, name="tot", tag="g")
                nc.tensor.matmul(
                    tot[0:1, 0:1], sumlog[:], onesf_sb[:], start=True, stop=True
                )
                part_sb = small.tile([1, 1], F32, name="part_sb", tag="part_sb")
                nc.vector.tensor_copy(part_sb[:], tot[0:1, 0:1])
                nc.sync.dma_start(partial[:], part_sb[:])

    nc.finalize()
    return nc


def _get_nc():
    if "nc" not in _CACHE:
        _CACHE["nc"] = _build()
    return _CACHE["nc"]


def _in_maps(x: np.ndarray) -> list[dict]:
    ident = np.eye(P, dtype=np.float32).astype(ml_dtypes.bfloat16)
    ebig = np.zeros((P, NT + 3 * P), dtype=np.float32)
    ebig[np.arange(P), 3 * P + np.arange(P)] = -DIAG_C
    ebig = ebig.astype(ml_dtypes.bfloat16)
    xbf = x.astype(ml_dtypes.bfloat16)
    maps = []
    for m in range(N_CORES):
        xrot = np.concatenate([xbf[m * ROWS :], xbf[: m * ROWS]], axis=0)
        maps.append(
            {
                "xt": np.ascontiguousarray(xrot.T),
                "ident": ident,
                "ebig": ebig,
            }
        )
    return maps


def run_kernel(x: np.ndarray, **spmd_kwargs):
    """Returns (loss_scalar_f32, BassKernelResults)."""
    res = run_bass_kernel_spmd(
        _get_nc(), _in_maps(x), core_ids=list(range(N_CORES)), **spmd_kwargs
    )
    s = sum(float(res.results[m]["partial"][0, 0]) for m in range(N_CORES))
    loss = np.float32(-0.5 * s / B)
    return np.asarray(loss, dtype=np.float32), res


def kernel(student_output: np.ndarray) -> np.ndarray:
    x = np.ascontiguousarray(np.asarray(student_output, dtype=np.float32))
    loss, _ = run_kernel(x)
    return loss


# revision 11
# speedup vs baseline: 1.3336x; 1.3336x over previous
"""KoLeo loss kernel for Trainium2 (8 NeuronCores, data-parallel rows).

reference semantics:
    x = l2_normalize(student_output)            # [B, D]
    dots = x @ x.T ; dots[i, i] = -1
    I = argmax(dots, 1)
    loss = -mean(log(||x - x[I] + eps|| + eps))

Since rows are unit-norm, ||x_i - x_j|| = sqrt(2 - 2 * dot(x_i, x_j)), so
    loss = -0.5 * mean(ln(2 - 2 * max_j!=i dots[i, j]))
(the eps terms contribute ~1e-8 relative and are dropped).

Sharding: each core gets the full x^T (bf16, host-cast), column-rotated so its
own 1024 rows come first, computes its [1024, 8192] slice of the gram matrix
in fp8-e4m3 with DoubleRow matmuls (2 K-planes per instruction), and reduces
to a scalar partial sum of ln(2 - 2*maxdot). Host sums the 8 partials.

The row-max drain is the second-largest cost after the matmuls (8.4M PSUM
f32 reads/core; DVE and ACT are the only PSUM readers, both 1 elem/cycle/
lane).  It is split:
  - 2 of 8 column groups: exact DVE reduce_max from PSUM.
  - 6 of 8: ACT smooth-max -- one fused Exp activation per PSUM tile with
    accum_out summing exp(beta*(dot-c)) along the row; the per-row max is
    recovered as c + ln(sum)/beta (beta=256, c=0.22: error ~1e-4 vs exact,
    and exp(256*(-64-c)) flushes to zero so the diagonal kill is free).
This puts ~55us of drain on the otherwise-idle ACT and keeps DVE under the
PE's ~75us of matmul streaming.

Other structure per 1024-wide column group:
  1. DMA x^T bf16 (host pre-casts f32->bf16: halves HBM traffic)
  2. xsq = x*x bf16 (GpSimd; DVE for group 0 to shorten the startup chain)
  3. column norms via bf16 ones-matmul (PE broadcasts sums across
     partitions); inv = exp(-0.5*ln(norm2)) (ACT; the act-table universe is
     pinned to natural_log_exp_and_others => exactly one ACT_TABLE_LOAD)
  4. normalize: xq = x * inv, bf16*bf16 -> fp8 DoubleRow planes (DVE)
  5. gram: per row-tile one [128,1024] PSUM tile, 4 DoubleRow matmuls
     (K=256 each); diagonal killed by one extra bf16 I.T@(-64 shifted)
  6. drain as above; epilogue folds smooth+direct maxes, ln(2-2*max),
     row-sum, partition-sum via f32 ones-matmul, scalar partial out
"""

import numpy as np
import ml_dtypes

import concourse.bacc as bacc
import concourse.hw_specs as hw_specs
import concourse.tile as tile
from concourse import mybir
from concourse.bass_utils import run_bass_kernel_spmd

B, D = 8192, 512
N_CORES = 8
ROWS = B // N_CORES          # 1024 rows per core
P = 128                      # SBUF partitions
KT = D // P                  # 4 contraction k-tiles
KB = KT // 2                 # 2 DoubleRow k-blocks (2 planes each)
M_TILES = ROWS // P          # 8 output row tiles
NT = 512                     # matmul moving free dim (psum bank)
GW = 1024                    # column-group width == gram PSUM tile (2 banks)
NG = B // GW                 # 8 column groups
DIAG_C = 64.0                # diagonal kill constant
N_WARM = 28                  # PE warm-up matmuls issued under the input DMA

BETA = 256.0                 # smooth-max sharpness
CSHIFT = 0.22                # smooth-max recentering constant
DIRECT_NG = (0, 3, 6)        # column groups drained by exact DVE reduce_max
N_DIRECT = len(DIRECT_NG)
N_EXP = NG - N_DIRECT
EXP_IDX = {ng: i for i, ng in enumerate(n for n in range(NG) if n not in DIRECT_NG)}

F32 = mybir.dt.float32
BF16 = mybir.dt.bfloat16
FP8 = mybir.dt.float8e4
AF = mybir.ActivationFunctionType
ALU = mybir.AluOpType
DR = mybir.MatmulPerfMode.DoubleRow

_CACHE: dict = {}


def _pin_act_tables():
    """Restrict the activation-table universe to natural_log_exp_and_others
    (contains ln, exp, copy, square, identity) so the table-load inserter
    emits exactly one ACT_TABLE_LOAD instead of thrashing ln<->exp sets.
    Set positions are preserved so the emitted act_func_set_id still indexes
    act_info.json correctly."""
    orig = hw_specs.get_activation_tables("gen3")
    pinned = {
        name: (fns if name == "natural_log_exp_and_others" else set())
        for name, fns in orig.items()
    }
    bacc.get_activation_tables = lambda arch: pinned


def _build():
    _pin_act_tables()
    nc = bacc.Bacc(
        "TRN2", target_bir_lowering=False, debug=False, num_devices=N_CORES
    )
    xt = nc.declare_dram_parameter("xt", [D, B], BF16, isOutput=False)
    ident = nc.declare_dram_parameter("ident", [P, P], BF16, isOutput=False)
    # ebig[p, 384 + p] = -DIAG_C, zero elsewhere; slicing [384-off : 896-off]
    # yields a [P, NT] tile with -DIAG_C at [p, off + p]
    ebig = nc.declare_dram_parameter("ebig", [P, NT + 3 * P], BF16, isOutput=False)
    partial = nc.declare_dram_parameter("partial", [1, 1], F32, isOutput=True)

    with tile.TileContext(nc) as tc:
        with (
            tc.tile_pool(name="big", bufs=1) as big,
            tc.tile_pool(name="work", bufs=2) as work,
            tc.tile_pool(name="sq", bufs=2) as sqp,
            tc.tile_pool(name="scr", bufs=3) as scrp,
            tc.tile_pool(name="small", bufs=2) as small,
        ):
            ident_sb = big.tile([P, P], BF16, name="ident_sb", tag="ident_sb")
            ebig_sb = big.tile([P, NT + 3 * P], BF16, name="ebig_sb", tag="ebig_sb")
            ones_sb = big.tile([P, P], BF16, name="ones_sb", tag="ones_sb")
            onesf_sb = big.tile([P, 1], F32, name="onesf_sb", tag="onesf_sb")
            nc.sync.dma_start(ident_sb[:], ident[:])
            nc.sync.dma_start(ebig_sb[:], ebig[:])
            nc.gpsimd.memset(ones_sb[:], 1.0)
            nc.gpsimd.memset(onesf_sb[:], 1.0)
            two_sb = small.tile([P, 1], F32, name="two_sb", tag="two_sb")
            nc.gpsimd.memset(two_sb[:], 2.0)
            # exp bias: exp(BETA*dot - BETA*CSHIFT)
            ebias_sb = small.tile([P, 1], F32, name="ebias_sb", tag="ebias_sb")
            nc.gpsimd.memset(ebias_sb[:], -BETA * CSHIFT)

            xbf = [
                big.tile([P, B], BF16, name=f"xbf{k}", tag=f"xbf{k}")
                for k in range(KT)
            ]
            xq = [
                big.tile([P, 2, B], FP8, name=f"xq{kb}", tag=f"xq{kb}")
                for kb in range(KB)
            ]
            invb = big.tile([P, B], BF16, name="invb", tag="invb")
            loglist = small.tile([P, M_TILES], F32, name="loglist", tag="loglist")
            # per (mi, direct group | smooth) row-max candidates
            maxall = small.tile(
                [P, M_TILES * (N_DIRECT + 1)], F32, name="maxall", tag="maxall"
            )
            # per (mi, exp group) partial exp-sums
            expall = small.tile(
                [P, M_TILES * N_EXP], F32, name="expall", tag="expall"
            )

            with (
                tc.tile_pool(name="npsum", bufs=1, space="PSUM") as npsum,
                tc.tile_pool(name="gpsum", bufs=3, space="PSUM") as gpsum,
            ):
                # issue ALL input DMAs upfront from the idle sync queue; the
                # xbf destinations are whole-tensor tiles with no recycling
                for ng in range(NG):
                    ns = slice(ng * GW, (ng + 1) * GW)
                    for k in range(KT):
                        nc.sync.dma_start(
                            xbf[k][:, ns], xt[k * P : (k + 1) * P, ns]
                        )

                # PE warm-up: keep the HAM activity window busy during the
                # initial DMA so gram matmuls run at 2.4 GHz from the start.
                warm = npsum.tile([P, GW], F32, name="warm", tag="nps")
                for _ in range(N_WARM):
                    nc.tensor.matmul(
                        warm[:, 0:P], ident_sb[:], ident_sb[:], start=True, stop=True
                    )

                def norm_stage(ng):
                    """xsq -> ones-matmul -> ln/exp -> normalize for group ng.
                    Emitted one group ahead, mid-way through the previous
                    group's gram loop, so the PE FIFO interleaves the norm
                    matmuls with gram matmuls instead of serializing at the
                    group boundary."""
                    ns = slice(ng * GW, (ng + 1) * GW)
                    xsq = [
                        sqp.tile([P, GW], BF16, name=f"xsq_{ng}_{k}", tag=f"xsq{k}")
                        for k in range(KT)
                    ]
                    sq_eng = nc.vector if ng == 0 else nc.gpsimd
                    for k in range(KT):
                        sq_eng.tensor_mul(xsq[k][:], xbf[k][:, ns], xbf[k][:, ns])
                    # column norms broadcast across partitions via ones-matmul
                    nps = npsum.tile([P, GW], F32, name="nps", tag="nps")
                    for c in range(GW // NT):
                        for k in range(KT):
                            nc.tensor.matmul(
                                nps[:, c * NT : (c + 1) * NT],
                                ones_sb[:],
                                xsq[k][:, c * NT : (c + 1) * NT],
                                start=(k == 0),
                                stop=(k == KT - 1),
                            )
                    # inv = exp(-0.5*ln(norm2)); one pinned table set
                    lntmp = work.tile([P, GW], F32, name="lntmp", tag="lntmp")
                    nc.scalar.activation(lntmp[:], nps[:], AF.Ln)
                    nc.scalar.activation(
                        invb[:, ns], lntmp[:], AF.Exp, scale=-0.5
                    )
                    # normalize into fp8 DoubleRow planes: xq = x * inv
                    for k in range(KT):
                        nc.vector.tensor_mul(
                            xq[k // 2][:, k % 2, ns], xbf[k][:, ns], invb[:, ns]
                        )

                norm_stage(0)
                for ng in range(NG):
                    # gram slice rows x this column group, then row-max drain
                    for mi in range(M_TILES):
                        if mi == 3 and ng + 1 < NG:
                            norm_stage(ng + 1)
                        g = gpsum.tile([P, GW], F32, name="g", tag="g")
                        # diag block for row-tile mi sits at columns
                        # [mi*128, mi*128+128) -- always group 0
                        diag_here = ng == 0
                        diag_c = (mi * P) // NT
                        for kb in range(KB):
                            for c in range(GW // NT):
                                c0 = ng * GW + c * NT
                                nc.tensor.matmul(
                                    g[:, c * NT : (c + 1) * NT],
                                    xq[kb][:, :, mi * P : (mi + 1) * P],
                                    xq[kb][:, :, c0 : c0 + NT],
                                    start=(kb == 0),
                                    stop=(
                                        kb == KB - 1
                                        and not (diag_here and c == diag_c)
                                    ),
                                    perf_mode=DR,
                                )
                        if diag_here:
                            off = (mi * P) % NT
                            # adds -DIAG_C at diag position [p, off+p]
                            nc.tensor.matmul(
                                g[:, diag_c * NT : (diag_c + 1) * NT],
                                ident_sb[:],
                                ebig_sb[:, 3 * P - off : 3 * P - off + NT],
                                start=False,
                                stop=True,
                            )
                        if ng in DIRECT_NG:
                            di = DIRECT_NG.index(ng)
                            col = mi * (N_DIRECT + 1) + di
                            nc.vector.reduce_max(
                                maxall[:, col : col + 1],
                                g[:],
                                axis=mybir.AxisListType.X,
                            )
                        else:
                            # fused smooth-max drain on ACT: accum_out sums
                            # exp(BETA*(dot - CSHIFT)) along the row
                            scr = scrp.tile([P, GW], BF16, name="scr", tag="scr")
                            col = mi * N_EXP + EXP_IDX[ng]
                            nc.scalar.activation(
                                scr[:],
                                g[:],
                                AF.Exp,
                                bias=ebias_sb[:],
                                scale=BETA,
                                accum_out=expall[:, col : col + 1],
                            )

                for mi in range(M_TILES):
                    # smooth max = CSHIFT + ln(sum of exp sums)/BETA
                    acc = small.tile([P, 1], F32, name="acc", tag="acc")
                    nc.vector.reduce_sum(
                        acc[:],
                        expall[:, mi * N_EXP : (mi + 1) * N_EXP],
                        axis=mybir.AxisListType.X,
                    )
                    acc2 = small.tile([P, 1], F32, name="acc2", tag="acc2")
                    nc.vector.tensor_scalar_max(acc2[:], acc[:], 1e-35)
                    lnacc = small.tile([P, 1], F32, name="lnacc", tag="lnacc")
                    nc.scalar.activation(lnacc[:], acc2[:], AF.Ln)
                    mcol = mi * (N_DIRECT + 1) + N_DIRECT
                    nc.vector.tensor_scalar(
                        maxall[:, mcol : mcol + 1],
                        lnacc[:],
                        1.0 / BETA,
                        CSHIFT,
                        op0=ALU.mult,
                        op1=ALU.add,
                    )
                    rowmax = small.tile([P, 1], F32, name="rowmax", tag="rowmax")
                    nc.vector.reduce_max(
                        rowmax[:],
                        maxall[:, mi * (N_DIRECT + 1) : (mi + 1) * (N_DIRECT + 1)],
                        axis=mybir.AxisListType.X,
                    )
                    # ln(2 - 2*maxdot) = 2*ln(nearest-neighbor distance)
                    nc.scalar.activation(
                        loglist[:, mi : mi + 1],
                        rowmax[:],
                        AF.Ln,
                        bias=two_sb[:],
                        scale=-2.0,
                    )

                # --- final reduction to one scalar per core ---
                sumlog = small.tile([P, 1], F32, name="sumlog", tag="sumlog")
                nc.vector.reduce_sum(
                    sumlog[:], loglist[:], axis=mybir.AxisListType.X
                )
                # partition sum via f32 matmul: [1,1] = sumlog.T @ ones
                tot = npsum.tile([P, GW], F32, name="tot", tag="nps")
                nc.tensor.matmul(
                    tot[0:1, 0:1], sumlog[:], onesf_sb[:], start=True, stop=True
                )
                part_sb = small.tile([1, 1], F32, name="part_sb", tag="part_sb")
                nc.vector.tensor_copy(part_sb[:], tot[0:1, 0:1])
                nc.sync.dma_start(partial[:], part_sb[:])

    nc.finalize()
    return nc


def _get_nc():
    if "nc" not in _CACHE:
        _CACHE["nc"] = _build()
    return _CACHE["nc"]


def _in_maps(x: np.ndarray) -> list[dict]:
    ident = np.eye(P, dtype=np.float32).astype(ml_dtypes.bfloat16)
    ebig = np.zeros((P, NT + 3 * P), dtype=np.float32)
    ebig[np.arange(P), 3 * P + np.arange(P)] = -DIAG_C
    ebig = ebig.astype(ml_dtypes.bfloat16)
    xbf = x.astype(ml_dtypes.bfloat16)
    maps = []
    for m in range(N_CORES):
        xrot = np.concatenate([xbf[m * ROWS :], xbf[: m * ROWS]], axis=0)
        maps.append(
            {
                "xt": np.ascontiguousarray(xrot.T),
                "ident": ident,
                "ebig": ebig,
            }
        )
    return maps


def run_kernel(x: np.ndarray, **spmd_kwargs):
    """Returns (loss_scalar_f32, BassKernelResults)."""
    res = run_bass_kernel_spmd(
        _get_nc(), _in_maps(x), core_ids=list(range(N_CORES)), **spmd_kwargs
    )
    s = sum(float(res.results[m]["partial"][0, 0]) for m in range(N_CORES))
    loss = np.float32(-0.5 * s / B)
    return np.asarray(loss, dtype=np.float32), res


def kernel(student_output: np.ndarray) -> np.ndarray:
    x = np.ascontiguousarray(np.asarray(student_output, dtype=np.float32))
    loss, _ = run_kernel(x)
    return loss
